# revision 6
# baseline (speedup 1.0000x reference)
# Trainium2 Bass kernel for CrossSpeakerAttention (v2, all-bf16 matmul path).
#
# Per-core (data-parallel over batch B=8 across 8 NeuronCores):
#   X = delta_u[b]  (T=1024, PD=512), heads H=8, D=64
#   token i attends to tokens j with  j < i  AND  spk[j] != spk[i]  AND valid[j].
#   out = softmax(QK^T/8 masked) V, concat heads, @ Wo.T + bo; fully-masked
#   rows produce exactly bo.
#
# Key differences vs v1:
#   - all matmul operands in bf16 (same PE rate as fp32r but no >=256 moving
#     constraint, half the DMA/copy bytes, 2x DVE copies).
#   - speaker/valid mask folded into the QK contraction as 5 extra rows of
#     per-head Q/K tiles (69-row contraction) -- no separate bias matmuls.
#   - scores for adjacent j-tile pairs land in one PSUM tile [128,2,512] and
#     are exp'd in one Activation instruction (fewer, larger exps).
#   - causal zeroing only on the exact 128-wide diagonal squares.
#   - deferred softmax normalization: denominators ride as V' ones-column;
#     reciprocals are broadcast across partitions with a one-hot matmul and
#     applied by DVE/Pool tensor_tensor into bf16 ot tiles.
#   - engine load spread across DVE and Pool; normalize/out-proj of i-block 0
#     overlaps attention of i-block 1.

import os
import sys
import numpy as np

sys.path.insert(0, "/opt/trn_rl_repo")

B, T, PD, H, D = 8, 1024, 512, 8, 64
NT = T // 128          # 8 j-tiles
NC_ = PD // 128        # 4 contraction tiles for projections
NIB = T // 512         # 2 i-blocks
BIG = 480.0            # additive mask before the 1/8 score scale -> -60
THR = 1e-12            # denominator floor
N_CORES = 8

_CACHE = {}


def _c0(ib, jt):
    return min(max(128 * jt - 512 * ib, 0), 512)


def _pairs(ib):
    kept = [jt for jt in range(NT) if _c0(ib, jt) < 512]
    return [tuple(kept[a : a + 2]) for a in range(0, len(kept), 2)]


def _build():
    import concourse.bass as bass
    import concourse.mybir as mybir
    import concourse.tile as tile
    from concourse import bacc

    f32 = mybir.dt.float32
    f32r = mybir.dt.float32r
    bf16 = mybir.dt.bfloat16
    AF = mybir.ActivationFunctionType
    OP = mybir.AluOpType

    nc = bacc.Bacc("TRN2", target_bir_lowering=False, debug=False)

    XT_d = nc.dram_tensor("XT", [PD, T], bf16, kind="ExternalInput")
    WqT_d = nc.dram_tensor("WqT", [PD, PD], bf16, kind="ExternalInput")
    WkT_d = nc.dram_tensor("WkT", [PD, PD], bf16, kind="ExternalInput")
    WvT_d = nc.dram_tensor("WvT", [PD, PD], bf16, kind="ExternalInput")
    WoT_d = nc.dram_tensor("WoT", [PD, PD], bf16, kind="ExternalInput")
    OHQ_d = nc.dram_tensor("OHQ", [5, T], bf16, kind="ExternalInput")
    OHK_d = nc.dram_tensor("OHK", [5, T], bf16, kind="ExternalInput")
    BO_d = nc.dram_tensor("BO", [1, PD], bf16, kind="ExternalInput")
    Y_d = nc.dram_tensor("Y", [T, PD], f32, kind="ExternalOutput")

    # simple alternation counters for DVE/Pool load balancing
    class Alt:
        def __init__(self, pattern):
            self.pattern = pattern  # list of engine pick fns
            self.i = 0

        def __call__(self):
            e = self.pattern[self.i % len(self.pattern)]
            self.i += 1
            return e

    with tile.TileContext(nc) as tc:
        import contextlib

        with contextlib.ExitStack() as ctx:
            const = ctx.enter_context(tc.tile_pool(name="const", bufs=1))

            # GPSIMD cannot access PSUM: every PSUM-touching op runs on DVE;
            # Pool gets the all-SBUF work (affine_select, partition_broadcast,
            # the normalize multiply).

            # ---- persistent SBUF tensors ----
            xt = const.tile([128, NC_, T], bf16, tag="xt")
            wq = const.tile([128, NC_, PD], bf16, tag="wq")
            wk = const.tile([128, NC_, PD], bf16, tag="wk")
            wv = const.tile([128, NC_, PD], bf16, tag="wv")
            wo = const.tile([128, NC_, PD], bf16, tag="wo")
            bo = const.tile([1, PD], bf16, tag="bo")
            onesb = const.tile([1, T], bf16, tag="onesb")
            qt = [const.tile([69, T], bf16, tag=f"qt{h}", name=f"qt{h}") for h in range(H)]
            kt = [const.tile([69, T], bf16, tag=f"kt{h}", name=f"kt{h}") for h in range(H)]
            vs = [const.tile([128, H, 66], bf16, tag=f"vs{t}", name=f"vs{t}") for t in range(NT)]
            ot = const.tile([128, NC_, T], bf16, tag="ot")
            dnm = [const.tile([8, 512], f32, tag=f"dnm{ib}", name=f"dnm{ib}") for ib in range(NIB)]
            dmx = [const.tile([8, 512], f32, tag=f"dmx{ib}", name=f"dmx{ib}") for ib in range(NIB)]
            rcpb = [const.tile([8, 512], bf16, tag=f"rcpb{ib}", name=f"rcpb{ib}") for ib in range(NIB)]
            rtmp = [const.tile([1, 512], bf16, tag=f"rtmp{i}", name=f"rtmp{i}") for i in range(16)]
            tmpd = [const.tile([1, 512], f32, tag=f"tmpd{i}", name=f"tmpd{i}") for i in range(16)]
            osb = [const.tile([64, 512], bf16, tag=f"osb{i}", name=f"osb{i}") for i in range(16)]
            wrm = const.tile([1, 16], f32, tag="wrm")

            # ---- input DMAs ----
            nc.sync.dma_start(xt[:], XT_d.ap().rearrange("(o p) t -> p o t", p=128))
            for w_sb, w_d in ((wq, WqT_d), (wk, WkT_d), (wv, WvT_d), (wo, WoT_d)):
                nc.sync.dma_start(w_sb[:], w_d.ap().rearrange("(o p) n -> p o n", p=128))
            for h in range(H):
                nc.sync.dma_start(qt[h][64:69, :], OHQ_d.ap())
                nc.sync.dma_start(kt[h][64:69, :], OHK_d.ap())
            nc.sync.dma_start(bo[:], BO_d.ap())
            nc.vector.memset(onesb[:], 1.0)
            for t in range(NT):
                nc.vector.memset(vs[t][:, :, 64:65], 1.0)
            # activation-table warmup (loads Exp table during the proj phase)
            nc.vector.memset(wrm[:], 0.0)
            nc.scalar.activation(wrm[:], wrm[:], AF.Exp, scale=1.0)

            # ---------------- projections ----------------
            with tc.tile_pool(name="pj", bufs=3, space="PSUM") as pj:
                # Q^T and K^T: [128 (head pair m), 512 t-block] -> per-head 64-row tiles
                for w_sb, dst in ((wq, qt), (wk, kt)):
                    for m in range(4):
                        for tb in range(NIB):
                            ps = pj.tile([128, 512], f32, tag="pj")
                            for k in range(NC_):
                                nc.tensor.matmul(
                                    ps[:],
                                    w_sb[:, k, 128 * m : 128 * (m + 1)],
                                    xt[:, k, 512 * tb : 512 * (tb + 1)],
                                    start=(k == 0),
                                    stop=(k == NC_ - 1),
                                )
                            isl = slice(512 * tb, 512 * (tb + 1))
                            nc.vector.tensor_copy(dst[2 * m][0:64, isl], ps[0:64, :])
                            nc.vector.tensor_copy(dst[2 * m + 1][0:64, isl], ps[64:128, :])
                # V: [128 t-tile, 512 dims] -> vs tiles + ones col
                for t in range(NT):
                    ps = pj.tile([128, 512], f32, tag="pj")
                    for k in range(NC_):
                        nc.tensor.matmul(
                            ps[:],
                            xt[:, k, 128 * t : 128 * (t + 1)],
                            wv[:, k, :],
                            start=(k == 0),
                            stop=(k == NC_ - 1),
                        )
                    nc.vector.tensor_copy(
                        vs[t][:, :, 0:64], ps[:].rearrange("p (h d) -> p h d", d=64)
                    )

            # ---------------- attention + normalize + out-proj ----------------
            with tc.tile_pool(name="s2", bufs=2, space="PSUM") as s2p, \
                 tc.tile_pool(name="ov", bufs=2, space="PSUM") as ovp, \
                 tc.tile_pool(name="fp", bufs=2, space="PSUM") as fpp, \
                 tc.tile_pool(name="rbs", bufs=2) as rbsp, \
                 tc.tile_pool(name="esb", bufs=3) as esb, \
                 tc.tile_pool(name="ysb", bufs=2) as ysb:

                def attn_head(ib, h):
                    i0 = 512 * ib
                    prs = _pairs(ib)
                    o_ps = ovp.tile([65, 512], f32, tag="ov", name=f"ov{ib}_{h}")
                    n_sl = sum(len(p) for p in prs)
                    sl_i = 0
                    for pr in prs:
                        s2 = s2p.tile([128, 2, 512], f32, tag="s2")
                        e2 = esb.tile([128, 2, 512], bf16, tag="e2")
                        cemin = _c0(ib, pr[0])
                        for s, jt in enumerate(pr):
                            # start both slots at the pair's cemin so the whole
                            # exp'd region is written (slot-1 extra cols are
                            # causally dead and never consumed by AV)
                            nc.tensor.matmul(
                                s2[:, s, cemin:512],
                                kt[h][:, 128 * jt : 128 * (jt + 1)],
                                qt[h][:, i0 + cemin : i0 + 512],
                                start=True,
                                stop=True,
                            )
                        nc.scalar.activation(
                            e2[:, :, cemin:512],
                            s2[:, :, cemin:512],
                            AF.Exp,
                            scale=0.125,
                        )
                        for s, jt in enumerate(pr):
                            if jt >= 4 * ib:  # diagonal square -> exact causal zeros
                                c0 = _c0(ib, jt)
                                sl = e2[:, s, c0 : c0 + 128]
                                nc.gpsimd.affine_select(
                                    sl,
                                    sl,
                                    pattern=[[1, 128]],
                                    base=0,
                                    channel_multiplier=-1,
                                    compare_op=OP.is_gt,
                                    fill=0.0,
                                )
                        for s, jt in enumerate(pr):
                            c0 = _c0(ib, jt)
                            nc.tensor.matmul(
                                o_ps[:, c0:512],
                                vs[jt][:, h, 0:65],
                                e2[:, s, c0:512],
                                start=(sl_i == 0),
                                stop=(sl_i == n_sl - 1),
                            )
                            sl_i += 1
                    # collect denominator row + stage O rows, then free o_ps
                    t_i = 8 * ib + h
                    nc.vector.tensor_copy(tmpd[t_i][:], o_ps[64:65, :])
                    nc.vector.tensor_copy(osb[t_i][:], o_ps[0:64, :])
                    nc.sync.dma_start(dnm[ib][h : h + 1, :], tmpd[t_i][:])

                def floor_recip(ib):
                    nc.vector.tensor_scalar(dmx[ib][:], dnm[ib][:], THR, None, OP.max)
                    with nc.allow_low_precision(reason="bf16 softmax reciprocal"):
                        nc.vector.reciprocal(rcpb[ib][:], dmx[ib][:])
                    for h in range(H):
                        nc.sync.dma_start(rtmp[8 * ib + h][:], rcpb[ib][h : h + 1, :])

                def norm_head(ib, h):
                    t_i = 8 * ib + h
                    rbc = rbsp.tile([64, 512], bf16, tag="rbc", name=f"rbc{ib}_{h}")
                    nc.gpsimd.partition_broadcast(rbc[:], rtmp[t_i][:])
                    nc.gpsimd.tensor_tensor(
                        ot[64 * (h % 2) : 64 * (h % 2) + 64, h // 2, 512 * ib : 512 * ib + 512],
                        osb[t_i][:],
                        rbc[:],
                        OP.mult,
                    )

                def outproj(t):
                    ps = fpp.tile([128, 512], f32, tag="fp", name=f"fp{t}")
                    for k in range(NC_):
                        nc.tensor.matmul(
                            ps[:],
                            ot[:, k, 128 * t : 128 * (t + 1)],
                            wo[:, k, :],
                            start=(k == 0),
                            stop=False,
                        )
                    nc.tensor.matmul(
                        ps[:],
                        onesb[0:1, 128 * t : 128 * (t + 1)],
                        bo[:],
                        start=False,
                        stop=True,
                    )
                    y = ysb.tile([128, 512], f32, tag="y")
                    nc.vector.tensor_copy(y[:], ps[:])
                    nc.sync.dma_start(Y_d.ap()[128 * t : 128 * (t + 1), :], y[:])

                # schedule: ib0 attn | ib1 attn h0-3 | ib0 norm | ib1 attn h4-7 |
                #           ib0 outproj | ib1 norm | ib1 outproj
                for h in range(H):
                    attn_head(0, h)
                floor_recip(0)
                for h in range(4):
                    attn_head(1, h)
                for h in range(H):
                    norm_head(0, h)
                for h in range(4, H):
                    attn_head(1, h)
                floor_recip(1)
                for t in range(4):
                    outproj(t)
                for h in range(H):
                    norm_head(1, h)
                for t in range(4, NT):
                    outproj(t)

    nc.compile()
    return nc


def _prep_core(b, delta_u, speaker_ids, valid_mask, WqT, WkT, WvT, WoT, bo):
    import ml_dtypes

    bf = ml_dtypes.bfloat16
    XT = np.ascontiguousarray(delta_u[b].T).astype(bf)
    spk = np.asarray(speaker_ids[b]).astype(np.int64)
    valid = np.asarray(valid_mask[b]).astype(np.float32)
    oh = np.zeros((4, T), dtype=np.float32)
    for s in range(4):
        oh[s] = (spk == s).astype(np.float32)
    OHQ = np.zeros((5, T), dtype=np.float32)
    OHK = np.zeros((5, T), dtype=np.float32)
    OHQ[0] = 1.0
    OHQ[1:5] = oh
    OHK[0] = -BIG * (1.0 - valid)
    OHK[1:5] = -BIG * oh
    return {
        "XT": XT,
        "WqT": WqT,
        "WkT": WkT,
        "WvT": WvT,
        "WoT": WoT,
        "OHQ": OHQ.astype(bf),
        "OHK": OHK.astype(bf),
        "BO": bo.reshape(1, PD).astype(bf),
    }


def kernel(**inputs) -> np.ndarray:
    import ml_dtypes
    from concourse.bass_utils import run_bass_kernel_spmd

    bf = ml_dtypes.bfloat16

    if "nc" not in _CACHE:
        _CACHE["nc"] = _build()
    nc = _CACHE["nc"]

    delta_u = np.asarray(inputs["delta_u"], dtype=np.float32)
    speaker_ids = np.asarray(inputs["speaker_ids"])
    valid_mask = np.asarray(inputs["valid_mask"])
    Wq = np.asarray(inputs["Wq"], dtype=np.float32)
    Wk = np.asarray(inputs["Wk"], dtype=np.float32)
    Wv = np.asarray(inputs["Wv"], dtype=np.float32)
    Wo = np.asarray(inputs["Wo"], dtype=np.float32)
    bo = np.asarray(inputs["bo"], dtype=np.float32)

    WqT = np.ascontiguousarray(Wq.T).astype(bf)
    WkT = np.ascontiguousarray(Wk.T).astype(bf)
    WvT = np.ascontiguousarray(Wv.T).astype(bf)
    WoT = np.ascontiguousarray(Wo.T).astype(bf)
    in_maps = [
        _prep_core(b, delta_u, speaker_ids, valid_mask, WqT, WkT, WvT, WoT, bo)
        for b in range(N_CORES)
    ]
    _CACHE["last_in_maps"] = in_maps
    res = run_bass_kernel_spmd(nc, in_maps, list(range(N_CORES)))
    out = np.stack([res.results[b]["Y"] for b in range(N_CORES)], axis=0)
    return out.astype(np.float32)


if __name__ == "__main__":
    rng = np.random.default_rng(0)
    ins = {
        "delta_u": rng.standard_normal((B, T, PD), dtype=np.float32),
        "speaker_ids": rng.integers(0, 4, size=(B, T)),
        "valid_mask": np.ones((B, T), dtype=bool),
        "Wq": rng.standard_normal((PD, PD), dtype=np.float32) * PD**-0.5,
        "Wk": rng.standard_normal((PD, PD), dtype=np.float32) * PD**-0.5,
        "Wv": rng.standard_normal((PD, PD), dtype=np.float32) * PD**-0.5,
        "Wo": rng.standard_normal((PD, PD), dtype=np.float32) * PD**-0.5,
        "bo": np.zeros((PD,), dtype=np.float32),
    }
    y = kernel(**ins)
    print("kernel ran, out shape", y.shape)
